# revision 1
# baseline (speedup 1.0000x reference)
"""Sliding-window causal self-attention (GQA + RoPE + QK-RMSNorm + ve-gate) on
8 Trainium2 NeuronCores.

Sharding: core c handles (batch b = c // 4, kv-head g = c % 4): data parallel
over batch x tensor parallel over the 4 KV head groups (4 query heads per
core). Each core computes its partial c_proj output; the all-reduce over the 4
head shards is a host-side sum.

Device design (per core):
  - x is fed transposed (xT: C x T) so all projections contract over the
    partition axis.
  - q, k are built transposed (qT/kT: head-dim x T); scores are computed
    TRANSPOSED (S^T: tk x tq) so softmax denominators come from a ones-matmul
    on the Tensor engine and P@V needs no transposes.
  - softmax skips max-subtraction: QK RMS-norm bounds |scores| <= 1.44*sqrt(128)
    so exp() cannot overflow in fp32. Masking is a -100 bias on the two
    triangular boundary blocks; masked weights underflow to 0.
  - k's rms-norm scale rides the per-partition `scale` operand of the Exp
    activation; q's rides the PSUM-evacuation multiply.
  - all matmuls run in float32r (full PE rate for moving dim >= 256,
    ~1.6e-4 matmul relerr vs fp32).
"""

import sys

sys.path.insert(0, "/opt/trn_rl_repo")

import numpy as np

B, T, C = 2, 2048, 2048
NH, NKV, HD = 16, 4, 128
GATE_CH = 12
HPC = NH // NKV          # q heads per core
TS = 512                 # token-slice width
NSL = T // TS            # 4 slices
NCK = C // 128           # 16 contraction chunks
TPS = TS // 128          # 4 token tiles per slice
NTT = T // 128           # 16 token tiles
EPS = 1e-6
NEG = -100.0

A_Q = 1.2 / np.sqrt(float(HD))   # rms-norm scale folded into q (incl 1/sqrt(HD))
A_K = 1.2                        # rms-norm scale folded into exp() scale arg
S_Q = float(1.0 / (HD * A_Q * A_Q))
B_Q = float(EPS / (A_Q * A_Q))
S_K = float(1.0 / (HD * A_K * A_K))
B_K = float(EPS / (A_K * A_K))

_compiled = {}


def _ktiles(m4, W):
    """k-tiles overlapping q-slice m4 with their valid tq-column extents.

    Returns list of (n, f0, f1, causal_block_col, edge_block_col); columns are
    relative to the slice (0..TS). First entry covers [0, TS) fully (it opens
    the PSUM accumulation group).
    """
    assert W % 128 == 0 and W >= 384
    out = []
    for n in range(0, TPS * m4 + TPS):
        f0 = max(0, 128 * n - TS * m4)
        f1 = min(TS, 128 * n + W + 128 - TS * m4)
        if f1 <= f0:
            continue
        causal = 128 * n >= TS * m4            # diagonal staircase inside tile
        edge = (128 * n + W + 128 - TS * m4) <= TS  # window lower edge inside
        cb = f0 if causal else None
        eb = (f1 - 128) if edge else None
        out.append((n, f0, f1, cb, eb))
    full = [e for e in out if e[1] == 0 and e[2] == TS]
    assert full, "need one full-extent tile to open the PSUM group"
    first = full[0]
    rest = [e for e in out if e[0] != first[0]]
    return [first] + rest


def _build(W):
    import concourse.bass as bass
    import concourse.tile as tile
    from concourse import bacc, mybir
    from concourse.masks import make_identity
    from contextlib import ExitStack

    f32 = mybir.dt.float32
    f32r = mybir.dt.float32r
    AF = mybir.ActivationFunctionType
    OP = mybir.AluOpType

    nc = bacc.Bacc(None, target_bir_lowering=False)

    xT = nc.dram_tensor("xT", [C, T], f32r, kind="ExternalInput")
    wq = nc.dram_tensor("wqT", [C, HPC * HD], f32r, kind="ExternalInput")
    wk = nc.dram_tensor("wkT", [C, HD], f32r, kind="ExternalInput")
    wv = nc.dram_tensor("wvT", [C, HD], f32r, kind="ExternalInput")
    wp = nc.dram_tensor("wpT", [HPC * HD, C], f32r, kind="ExternalInput")
    wgd = nc.dram_tensor("wg", [GATE_CH, 1], f32r, kind="ExternalInput")
    ccd = nc.dram_tensor("cc", [HD, T], f32, kind="ExternalInput")
    ssd = nc.dram_tensor("ss", [HD, T], f32, kind="ExternalInput")
    ved = nc.dram_tensor("ve", [T, HD], f32, kind="ExternalInput")
    btrid = nc.dram_tensor("btri", [128, 128], f32, kind="ExternalInput")
    etrid = nc.dram_tensor("etri", [128, 128], f32, kind="ExternalInput")
    outT = nc.dram_tensor("outT", [C, T], f32, kind="ExternalOutput")

    with tile.TileContext(nc) as tc, ExitStack() as ctx:
        res = ctx.enter_context(tc.tile_pool(name="res", bufs=1))
        xc_p = ctx.enter_context(tc.tile_pool(name="xc", bufs=1))
        tab_p = ctx.enter_context(tc.tile_pool(name="tab", bufs=1))
        work_p = ctx.enter_context(tc.tile_pool(name="work", bufs=2))
        sq_p = ctx.enter_context(tc.tile_pool(name="sq", bufs=3))
        bc_p = ctx.enter_context(tc.tile_pool(name="bc", bufs=2))
        qt_p = ctx.enter_context(tc.tile_pool(name="qt", bufs=2))
        es_p = ctx.enter_context(tc.tile_pool(name="es", bufs=4))
        yt_p = ctx.enter_context(tc.tile_pool(name="yt", bufs=1))
        ot_p = ctx.enter_context(tc.tile_pool(name="ot", bufs=3))
        row_p = ctx.enter_context(tc.tile_pool(name="rows", bufs=1))

        ps_qkv = ctx.enter_context(tc.tile_pool(name="ps_qkv", bufs=2, space="PSUM"))
        ps_s = ctx.enter_context(tc.tile_pool(name="ps_s", bufs=3, space="PSUM"))
        ps_row = ctx.enter_context(tc.tile_pool(name="ps_row", bufs=3, space="PSUM"))
        dram_p = ctx.enter_context(tc.tile_pool(name="dram", bufs=2, space="DRAM"))

        # resident tensors; weight loads split per chunk so the first QKV
        # matmuls can start as soon as their chunk lands (startup latency).
        wq_sb = res.tile([128, NCK, HPC * HD], f32r)
        wk_sb = res.tile([128, NCK, HD], f32r)
        wv_sb = res.tile([128, NCK, HD], f32r)
        wp_sb = res.tile([128, HPC, C], f32r)   # loaded later, before cproj(0)
        wg_sb = res.tile([GATE_CH, 1], f32r)
        nc.sync.dma_start(out=wg_sb, in_=wgd[:, :])
        btri_sb = res.tile([128, 128], f32)
        nc.sync.dma_start(out=btri_sb, in_=btrid[:, :])
        etri_sb = res.tile([128, 128], f32)
        nc.sync.dma_start(out=etri_sb, in_=etrid[:, :])
        ident = res.tile([128, 128], f32)
        make_identity(nc, ident)
        ones_f = res.tile([128, 1], f32)
        nc.vector.memset(ones_f, 1.0)
        ones_sb = ones_f.bitcast(f32r)
        bq_sb = res.tile([1, 1], f32)
        nc.vector.memset(bq_sb, B_Q)
        bk_sb = res.tile([128, 1], f32)
        nc.vector.memset(bk_sb, B_K)
        kT_sb = res.tile([128, T], f32r)        # rotated k, head-dim on partitions
        vn_sb = res.tile([128, NTT, HD], f32r)  # v natural, token tiles on partitions
        rnk_sb = res.tile([128, NTT], f32)      # per-k-tile rms-norm columns

        def rope_inplace(dst, cc_sl, ss_sl):
            """dst (128, TS) f32r holding pre-rotation values. In-place RoPE."""
            qsw = work_p.tile([128, TS], f32, tag="qsw")
            nc.sync.dma_start(out=qsw[0:64, :], in_=dst[64:128, :].bitcast(f32))
            nc.sync.dma_start(out=qsw[64:128, :], in_=dst[0:64, :].bitcast(f32))
            tmp = work_p.tile([128, TS], f32, tag="tmp")
            nc.gpsimd.tensor_mul(tmp, qsw, ss_sl)
            nc.vector.tensor_mul(dst, dst.bitcast(f32), cc_sl)
            nc.vector.tensor_add(dst, dst.bitcast(f32), tmp)

        for m4 in range(NSL):
            t0 = m4 * TS
            # ---- stream x slice + tables ----
            xc = []
            for c in range(NCK):
                xt = xc_p.tile([128, TS], f32r, tag=f"xc{c}")
                nc.sync.dma_start(out=xt, in_=xT[c * 128:(c + 1) * 128, t0:t0 + TS])
                xc.append(xt)
                if m4 == 0:
                    nc.sync.dma_start(out=wk_sb[:, c, :],
                                      in_=wk[c * 128:(c + 1) * 128, :])
            cc_sl = tab_p.tile([128, TS], f32, tag="cc")
            nc.sync.dma_start(out=cc_sl, in_=ccd[:, t0:t0 + TS])
            ss_sl = tab_p.tile([128, TS], f32, tag="ss")
            nc.sync.dma_start(out=ss_sl, in_=ssd[:, t0:t0 + TS])
            ve_sl = tab_p.tile([128, TPS, HD], f32, tag="ve")
            nc.sync.dma_start(
                out=ve_sl, in_=ved[t0:t0 + TS, :].rearrange("(tt p) h -> p tt h", p=128)
            )

            # ---- gate columns: 3*sigmoid(x[:, :12] @ wg) ----
            ps_g = ps_row.tile([1, TS], f32, tag="rows")
            nc.tensor.matmul(ps_g, wg_sb, xc[0][0:GATE_CH, :], start=True, stop=True)
            g_row = row_p.tile([1, TS], f32, tag="grow")
            nc.scalar.activation(g_row, ps_g, AF.Exp, scale=-1.0)
            nc.vector.tensor_scalar(out=g_row, in0=g_row, scalar1=1.0, scalar2=None,
                                    op0=OP.add)
            nc.vector.reciprocal(g_row, g_row)
            g_dr = dram_p.tile([TS], f32, tag="gdr")
            nc.sync.dma_start(out=g_dr, in_=g_row)
            gate_c = row_p.tile([128, TPS], f32, tag="gate")
            nc.sync.dma_start(
                out=gate_c,
                in_=bass.AP(tensor=g_dr.tensor, offset=g_dr.offset,
                            ap=[[1, 128], [128, TPS]]),
            )

            # ---- k projection + rms-norm cols + rope ----
            ps_k = ps_qkv.tile([128, TS], f32, tag="qkv")
            for c in range(NCK):
                nc.tensor.matmul(ps_k, wk_sb[:, c, :], xc[c],
                                 start=(c == 0), stop=(c == NCK - 1))
            sq_k = sq_p.tile([128, TS], f32r, tag="sq")
            nc.scalar.activation(sq_k, ps_k, AF.Square)
            ps_rk = ps_row.tile([1, TS], f32, tag="rows")
            nc.tensor.matmul(ps_rk, ones_sb, sq_k, start=True, stop=True)
            srk = row_p.tile([1, TS], f32, tag="srk")
            nc.scalar.activation(srk, ps_rk, AF.Ln, bias=bk_sb[0:1], scale=S_K)
            nc.scalar.activation(srk, srk, AF.Exp, scale=-0.5)
            k_dr = dram_p.tile([TS], f32, tag="kdr")
            nc.sync.dma_start(out=k_dr, in_=srk)
            nc.sync.dma_start(
                out=rnk_sb[:, m4 * TPS:(m4 + 1) * TPS],
                in_=bass.AP(tensor=k_dr.tensor, offset=k_dr.offset,
                            ap=[[1, 128], [128, TPS]]),
            )
            k_sl = kT_sb[:, t0:t0 + TS]
            nc.vector.tensor_copy(k_sl, ps_k)
            rope_inplace(k_sl, cc_sl, ss_sl)

            # ---- v projection + transpose to natural + gate-add ----
            if m4 == 0:
                for c in range(NCK):
                    nc.sync.dma_start(out=wv_sb[:, c, :],
                                      in_=wv[c * 128:(c + 1) * 128, :])
            ps_v = ps_qkv.tile([128, TS], f32, tag="qkv")
            for c in range(NCK):
                nc.tensor.matmul(ps_v, wv_sb[:, c, :], xc[c],
                                 start=(c == 0), stop=(c == NCK - 1))
            vT_s = work_p.tile([128, TS], f32, tag="qsw")
            nc.vector.tensor_copy(vT_s, ps_v)
            for tt in range(TPS):
                ps_t = ps_s.tile([128, TS], f32, tag="s")
                nc.tensor.transpose(ps_t[:, 0:128], vT_s[:, tt * 128:(tt + 1) * 128],
                                    ident)
                gtmp = work_p.tile([128, HD], f32, tag="gtmp")
                nc.vector.tensor_scalar(out=gtmp, in0=ve_sl[:, tt, :],
                                        scalar1=gate_c[:, tt:tt + 1], scalar2=3.0,
                                        op0=OP.mult, op1=OP.mult)
                nc.vector.tensor_add(vn_sb[:, m4 * TPS + tt, :], ps_t[:, 0:128], gtmp)

            # ---- q projections (4 heads) + rms-norm + rope ----
            if m4 == 0:
                for c in range(NCK):
                    nc.sync.dma_start(out=wq_sb[:, c, :],
                                      in_=wq[c * 128:(c + 1) * 128, :])
            qts = []
            for h in range(HPC):
                ps_q = ps_qkv.tile([128, TS], f32, tag="qkv")
                for c in range(NCK):
                    nc.tensor.matmul(ps_q, wq_sb[:, c, h * HD:(h + 1) * HD], xc[c],
                                     start=(c == 0), stop=(c == NCK - 1))
                sq_q = sq_p.tile([128, TS], f32r, tag="sq")
                nc.scalar.activation(sq_q, ps_q, AF.Square)
                ps_r = ps_row.tile([1, TS], f32, tag="rows")
                nc.tensor.matmul(ps_r, ones_sb, sq_q, start=True, stop=True)
                srow = row_p.tile([1, TS], f32, tag="srow")
                nc.scalar.activation(srow, ps_r, AF.Ln, bias=bq_sb, scale=S_Q)
                nc.scalar.activation(srow, srow, AF.Exp, scale=-0.5)
                rbc = bc_p.tile([128, TS], f32, tag="bc")
                nc.gpsimd.partition_broadcast(rbc, srow)
                qt = qt_p.tile([128, TS], f32r, tag=f"qt{h}")
                nc.vector.tensor_mul(qt, ps_q, rbc)
                rope_inplace(qt, cc_sl, ss_sl)
                qts.append(qt)

            # ---- attention (scores transposed: tk on partitions, tq free) ----
            tiles = _ktiles(m4, W)
            last = len(tiles) - 1
            yts = []
            for h in range(HPC):
                ps_out = ps_row.tile([128, TS], f32, tag="rows")
                ps_sum = ps_row.tile([1, TS], f32, tag="rows")
                for idx, (n, f0, f1, cb, eb) in enumerate(tiles):
                    pss = ps_s.tile([128, TS], f32, tag="s")
                    nc.tensor.matmul(pss[:, f0:f1], kT_sb[:, n * 128:(n + 1) * 128],
                                     qts[h][:, f0:f1], start=True, stop=True)
                    es = es_p.tile([128, TS], f32r, tag="es")
                    nc.scalar.activation(es[:, f0:f1], pss[:, f0:f1], AF.Exp,
                                         scale=rnk_sb[:, n:n + 1])
                    if cb is not None:
                        nc.gpsimd.tensor_mul(es[:, cb:cb + 128],
                                             es[:, cb:cb + 128].bitcast(f32), btri_sb)
                    if eb is not None:
                        nc.gpsimd.tensor_mul(es[:, eb:eb + 128],
                                             es[:, eb:eb + 128].bitcast(f32), etri_sb)
                    nc.tensor.matmul(ps_sum[:, f0:f1], ones_sb, es[:, f0:f1],
                                     start=(idx == 0), stop=(idx == last))
                    nc.tensor.matmul(ps_out[:, f0:f1], vn_sb[:, n, :], es[:, f0:f1],
                                     start=(idx == 0), stop=(idx == last))
                rsum = row_p.tile([1, TS], f32, tag="rsum")
                nc.vector.reciprocal(rsum, ps_sum)
                sbc = bc_p.tile([128, TS], f32, tag="bc")
                nc.gpsimd.partition_broadcast(sbc, rsum)
                yt = yt_p.tile([128, TS], f32r, tag=f"yt{h}")
                nc.vector.tensor_mul(yt, ps_out, sbc)
                yts.append(yt)

            # ---- c_proj partial: outT[co, t] = sum_h wpT[h].T @ yT[h] ----
            if m4 == 0:
                for h in range(HPC):
                    nc.sync.dma_start(out=wp_sb[:, h, :],
                                      in_=wp[h * 128:(h + 1) * 128, :])
            for co in range(NTT):
                ps_p = ps_s.tile([128, TS], f32, tag="s")
                for h in range(HPC):
                    nc.tensor.matmul(ps_p, wp_sb[:, h, co * 128:(co + 1) * 128],
                                     yts[h], start=(h == 0), stop=(h == HPC - 1))
                ot = ot_p.tile([128, TS], f32, tag="ot")
                nc.vector.tensor_copy(ot, ps_p)
                nc.sync.dma_start(out=outT[co * 128:(co + 1) * 128, t0:t0 + TS],
                                  in_=ot)

    # Restrict the activation-table picker to the one set containing every
    # ACT function we use (exp, ln, square, copy, identity): without this the
    # greedy picker alternates exp_and_others <-> natural_log, inserting a
    # ~1.3us table load per switch. Set ids are positions in act_info.json's
    # list, so unwanted sets are emptied rather than removed.
    import concourse.hw_specs as hw_specs
    import concourse.bacc as bacc_mod

    orig = hw_specs.get_activation_tables

    def only_combined(arch):
        t = orig(arch)
        return {k: (v if k == "natural_log_exp_and_others" else set())
                for k, v in t.items()}

    hw_specs.get_activation_tables = only_combined
    bacc_mod.get_activation_tables = only_combined
    try:
        nc.compile()
    finally:
        hw_specs.get_activation_tables = orig
        bacc_mod.get_activation_tables = orig
    return nc


def _prep_inputs(x, ve, cos, sin, Wq, Wk, Wv, Wproj, Wgate, W):
    cosT = np.ascontiguousarray(cos[0, :, 0, :].T)  # (64, T)
    sinT = np.ascontiguousarray(sin[0, :, 0, :].T)
    cc = np.concatenate([cosT, cosT], axis=0).astype(np.float32)
    ss = np.concatenate([sinT, -sinT], axis=0).astype(np.float32)
    p = np.arange(128)[:, None]
    f = np.arange(128)[None, :]
    btri = (p <= f).astype(np.float32)
    etri = (f <= p + (W % 128)).astype(np.float32)

    in_maps = []
    for core in range(8):
        b, g = core // NKV, core % NKV
        hs = slice(g * HPC * HD, (g + 1) * HPC * HD)
        ks = slice(g * HD, (g + 1) * HD)
        in_maps.append({
            "xT": np.ascontiguousarray(x[b].T),
            "wqT": np.ascontiguousarray(Wq[hs, :].T),
            "wkT": np.ascontiguousarray(Wk[ks, :].T),
            "wvT": np.ascontiguousarray(Wv[ks, :].T),
            "wpT": np.ascontiguousarray(Wproj[:, hs].T),
            "wg": np.ascontiguousarray(Wgate[g][:, None]),
            "cc": cc,
            "ss": ss,
            "ve": np.ascontiguousarray(ve[b][:, ks]),
            "btri": btri,
            "etri": etri,
        })
    return in_maps


def _run(inputs, trace=False):
    from concourse.bass_utils import run_bass_kernel_spmd

    x = np.asarray(inputs["x"], dtype=np.float32)
    ve = np.asarray(inputs["ve"], dtype=np.float32)
    cos = np.asarray(inputs["cos"], dtype=np.float32)
    sin = np.asarray(inputs["sin"], dtype=np.float32)
    Wq = np.asarray(inputs["Wq"], dtype=np.float32)
    Wk = np.asarray(inputs["Wk"], dtype=np.float32)
    Wv = np.asarray(inputs["Wv"], dtype=np.float32)
    Wproj = np.asarray(inputs["Wproj"], dtype=np.float32)
    Wgate = np.asarray(inputs["Wgate"], dtype=np.float32)
    W = int(inputs["window_size"])

    if W not in _compiled:
        _compiled[W] = _build(W)
    nc = _compiled[W]

    in_maps = _prep_inputs(x, ve, cos, sin, Wq, Wk, Wv, Wproj, Wgate, W)
    res = run_bass_kernel_spmd(nc, in_maps, core_ids=list(range(8)), trace=trace)

    out = np.zeros((B, T, C), dtype=np.float32)
    for core in range(8):
        b = core // NKV
        out[b] += res.results[core]["outT"].T
    return out, res


def kernel(**inputs):
    out, _ = _run(inputs, trace=False)
    return out



# revision 6
# speedup vs baseline: 1.0838x; 1.0838x over previous
"""Sliding-window causal self-attention (GQA + RoPE + QK-RMSNorm + ve-gate) on
8 Trainium2 NeuronCores.

Sharding: core c handles (batch b = c // 4, kv-head g = c % 4): data parallel
over batch x tensor parallel over the 4 KV head groups (4 query heads per
core). Each core computes its partial c_proj output; the all-reduce over the 4
head shards is a host-side sum.

Device design (per core), v2:
  - QKV projections run in fp8e4m3 DoubleRow with hi+lo split precision:
    x = xh + xl, W = Wh + Wl (each e4m3, pre-scaled by 16/64 into e4m3's
    normal range); the matmul computes xh*Wh + xh*Wl + xl*Wh as chunk-paired
    DoubleRow instructions (24 insts per 16-chunk contraction, 0.5 cyc/col
    each) -- ~bf16 accuracy at 0.75x the PE cost.
  - v is projected directly into natural (token-partition) layout by swapping
    the matmul operands (stationary = x tile, moving = Wv), killing the
    per-tile transposes.
  - rms-norm row sums use an all-ones [128,128] stationary so the row comes
    out replicated across all partitions; Ln/Exp activations produce the
    normalization factors as full [128,512] broadcast tiles directly (no
    gpsimd partition_broadcast anywhere).
  - k's norm is folded into the kT evacuation multiply, so the attention exp
    has no per-partition scale, allowing one wide ACT exp per PAIR of score
    tiles ([128, 2, ext] PSUM reads across 2 banks).
  - masking is done by accumulating -100 triangular bias matmuls into the
    scores PSUM group (Tensor engine), not by post-exp multiplies.
  - scores are computed transposed (tk on partitions) in bf16; softmax
    denominators come from all-ones matmuls (replicated), normalization and
    evacuations ride DVE; output tiles are evacuated bf16 in [128,1024] pairs
    and upcast host-side.
"""

import sys

sys.path.insert(0, "/opt/trn_rl_repo")

import numpy as np
import ml_dtypes

B, T, C = 2, 2048, 2048
NH, NKV, HD = 16, 4, 128
GATE_CH = 12
HPC = NH // NKV          # q heads per core
TS = 512                 # token-slice width
NSL = T // TS            # 4 slices
NCK = C // 128           # 16 contraction chunks
NPR = NCK // 2           # 8 chunk pairs
TPS = TS // 128          # 4 token tiles per slice
NTT = T // 128           # 16 token tiles
EPS = 1e-6
NEG = -100.0

SX = 16.0                # fp8 pre-scale on x
SW = 64.0                # fp8 pre-scale on weights
DQ = 1.0 / (SX * SW)     # descale of fp8 matmul products
SDEN = 1.0 / 16.0        # denominator pre-scale (folded into ones tile)

A_Q = 1.2 / np.sqrt(float(HD))   # q rms-norm scale (incl 1/sqrt(HD))
A_K = 1.2                        # k rms-norm scale
S_Q = float(1.0 / (HD * A_Q * A_Q))
B_Q = float((SX * SW) ** 2 * EPS / (A_Q * A_Q))
S_K = float(1.0 / (HD * A_K * A_K))
B_K = float((SX * SW) ** 2 * EPS / (A_K * A_K))

E4np = ml_dtypes.float8_e4m3
BFnp = ml_dtypes.bfloat16

_compiled = {}


def _ktiles(m4, W):
    """k-tiles overlapping q-slice m4 with their valid tq-column extents.

    Returns list of (n, f0, f1, cb, eb); columns relative to the slice. First
    entry covers [0, TS) fully (it opens the PSUM accumulation groups).
    """
    assert W % 128 == 0 and W >= 384
    out = []
    for n in range(0, TPS * m4 + TPS):
        f0 = max(0, 128 * n - TS * m4)
        f1 = min(TS, 128 * n + W + 128 - TS * m4)
        if f1 <= f0:
            continue
        causal = 128 * n >= TS * m4
        edge = (128 * n + W + 128 - TS * m4) <= TS
        cb = f0 if causal else None
        eb = (f1 - 128) if edge else None
        out.append((n, f0, f1, cb, eb))
    full = [e for e in out if e[1] == 0 and e[2] == TS]
    assert full, "need one full-extent tile to open the PSUM group"
    first = full[0]
    rest = [e for e in out if e[0] != first[0]]
    return [first] + rest


def _pairs(m4, W):
    """Pair the k-tiles; each pair shares one [128, 2, TS] PSUM tile and one
    wide exp. Returns list of (members, u0, u1) where members is a list of
    1 or 2 (slot, n, f0, f1, cb, eb) and [u0, u1) the union extent."""
    tiles = _ktiles(m4, W)
    pairs = []
    i = 0
    while i < len(tiles):
        mem = tiles[i:i + 2]
        u0 = min(e[1] for e in mem)
        u1 = max(e[2] for e in mem)
        pairs.append((
            [(s,) + e for s, e in enumerate(mem)], u0, u1))
        i += 2
    return pairs


def _build(W):
    import concourse.bass as bass
    import concourse.tile as tile
    from concourse import bacc, mybir
    from contextlib import ExitStack

    f32 = mybir.dt.float32
    bf16 = mybir.dt.bfloat16
    f8 = mybir.dt.float8e4
    AF = mybir.ActivationFunctionType
    OP = mybir.AluOpType
    DR = mybir.MatmulPerfMode.DoubleRow

    nc = bacc.Bacc(None, target_bir_lowering=False)

    xhd = nc.dram_tensor("xh", [C, T], f8, kind="ExternalInput")
    xld = nc.dram_tensor("xl", [C, T], f8, kind="ExternalInput")
    wqd = nc.dram_tensor("wq8", [C, 2, HPC * HD], f8, kind="ExternalInput")
    wkd = nc.dram_tensor("wk8", [C, 2, HD], f8, kind="ExternalInput")
    wvd = nc.dram_tensor("wv8", [C, 2, HD], f8, kind="ExternalInput")
    wpd = nc.dram_tensor("wpT", [HPC * HD, C], bf16, kind="ExternalInput")
    wgd = nc.dram_tensor("wg", [GATE_CH, 1], f8, kind="ExternalInput")
    ccd = nc.dram_tensor("cc", [HD, T], bf16, kind="ExternalInput")
    ssd = nc.dram_tensor("ss", [HD, T], bf16, kind="ExternalInput")
    ved = nc.dram_tensor("ve", [T, HD], bf16, kind="ExternalInput")
    btrid = nc.dram_tensor("btri", [128, 128], bf16, kind="ExternalInput")
    etrid = nc.dram_tensor("etri", [128, 128], bf16, kind="ExternalInput")
    identd = nc.dram_tensor("identb", [128, 128], bf16, kind="ExternalInput")
    fnegd = nc.dram_tensor("fneg", [128, 128], bf16, kind="ExternalInput")
    outT = nc.dram_tensor("outT", [C, T], bf16, kind="ExternalOutput")

    with tile.TileContext(nc) as tc, ExitStack() as ctx:
        res = ctx.enter_context(tc.tile_pool(name="res", bufs=1))
        xp_p = ctx.enter_context(tc.tile_pool(name="xp", bufs=2))
        tab_p = ctx.enter_context(tc.tile_pool(name="tab", bufs=2))
        sq_p = ctx.enter_context(tc.tile_pool(name="sq", bufs=2))
        nrm_p = ctx.enter_context(tc.tile_pool(name="nrm", bufs=2))
        qt_p = ctx.enter_context(tc.tile_pool(name="qt", bufs=2))
        wk_p = ctx.enter_context(tc.tile_pool(name="wkk", bufs=2))
        es_p = ctx.enter_context(tc.tile_pool(name="es", bufs=4))
        sbc_p = ctx.enter_context(tc.tile_pool(name="sbc", bufs=2))
        y_p = ctx.enter_context(tc.tile_pool(name="yy", bufs=2))
        ot_p = ctx.enter_context(tc.tile_pool(name="ot", bufs=3))
        row_p = ctx.enter_context(tc.tile_pool(name="rows", bufs=2))

        # PSUM: psS 2x[128,2,TS] (4 banks) for score pairs + cproj pairs;
        # psQ 2x[128,TS] (2 banks) for projections / norm rows; psO + psD.
        psS = ctx.enter_context(tc.tile_pool(name="psS", bufs=2, space="PSUM"))
        psQ = ctx.enter_context(tc.tile_pool(name="psQ", bufs=2, space="PSUM"))
        psO = ctx.enter_context(tc.tile_pool(name="psO", bufs=1, space="PSUM"))
        psD = ctx.enter_context(tc.tile_pool(name="psD", bufs=1, space="PSUM"))
        dram_p = ctx.enter_context(tc.tile_pool(name="dram", bufs=2, space="DRAM"))

        # ---- resident tensors ----
        wq_sb = res.tile([128, NCK, 2, HPC * HD], f8)
        wk_sb = res.tile([128, NCK, 2, HD], f8)
        wv_sb = res.tile([128, NCK, 2, HD], f8)
        wp_sb = res.tile([128, HPC, C], bf16)
        wg_sb = res.tile([GATE_CH, 1], f8)
        nc.sync.dma_start(out=wg_sb, in_=wgd[:, :])
        btri_sb = res.tile([128, 128], bf16)
        nc.sync.dma_start(out=btri_sb, in_=btrid[:, :])
        etri_sb = res.tile([128, 128], bf16)
        nc.sync.dma_start(out=etri_sb, in_=etrid[:, :])
        fneg_sb = res.tile([128, 128], bf16)
        nc.sync.dma_start(out=fneg_sb, in_=fnegd[:, :])
        ident = res.tile([128, 128], bf16)
        nc.sync.dma_start(out=ident, in_=identd[:, :])
        ones_sq = res.tile([128, 128], bf16)
        nc.vector.memset(ones_sq, 1.0)
        bq_sb = res.tile([128, 1], f32)
        nc.vector.memset(bq_sb, B_Q)
        bk_sb = res.tile([128, 1], f32)
        nc.vector.memset(bk_sb, B_K)
        kT_sb = res.tile([128, T], bf16)        # rotated+normed k, hd on partitions
        vn_sb = res.tile([128, NTT, HD], bf16)  # v natural, token tiles on partitions

        def rope_inplace(dst, cc_sl, ss_sl, eng):
            """dst (128, TS) bf16; in-place RoPE. halves swapped via SBUF-SBUF
            DMA (walrus requires same start partition on tensor_tensor); ss
            table carries [sin, -sin]."""
            sw = wk_p.tile([128, TS], bf16, tag="sw")
            nc.sync.dma_start(out=sw[0:64, :], in_=dst[64:128, :])
            nc.sync.dma_start(out=sw[64:128, :], in_=dst[0:64, :])
            eng.tensor_mul(sw, sw, ss_sl)
            eng.tensor_mul(dst, dst, cc_sl)
            eng.tensor_add(dst, dst, sw)

        def hilo_matmuls(ps, wsb, msl, xhp, xlp, nfree):
            """Accumulate sum_c x_c^T W_c over NCK chunks into ps using the
            hi-lo 3-product DoubleRow scheme. wsb: [128, NCK, 2, M] fp8 tile;
            msl: slice of last dim; xhp/xlp: lists of NPR [128,2,TS] tiles;
            nfree: moving col slice."""
            n_inst = 3 * NPR
            i = 0
            for j in range(NPR):
                for hl, xt in ((0, xhp[j]), (1, xhp[j]), (0, xlp[j])):
                    nc.tensor.matmul(
                        ps, wsb[:, 2 * j:2 * j + 2, hl, msl], xt[:, :, nfree],
                        start=(i == 0), stop=(i == n_inst - 1), perf_mode=DR)
                    i += 1

        def hilo_matmuls_vnat(ps, xhp, xlp, j_t, wsb):
            """v natural: stationary = x pair tiles (col block j_t), moving =
            Wv chunk pairs."""
            n_inst = 3 * NPR
            i = 0
            for j in range(NPR):
                for hl, xt in ((0, xhp[j]), (1, xhp[j]), (0, xlp[j])):
                    st = xt[:, :, j_t * 128:(j_t + 1) * 128]
                    mv = wsb[:, 2 * j:2 * j + 2, hl, :]
                    nc.tensor.matmul(ps, st, mv, start=(i == 0),
                                     stop=(i == n_inst - 1), perf_mode=DR)
                    i += 1

        for m4 in range(NSL):
            t0 = m4 * TS
            # ---- stream x slice (hi/lo chunk-pair tiles) + tables ----
            xhp, xlp = [], []
            for j in range(NPR):
                xh_t = xp_p.tile([128, 2, TS], f8, tag=f"xh{j}")
                nc.sync.dma_start(
                    out=xh_t,
                    in_=xhd[256 * j:256 * (j + 1), t0:t0 + TS].rearrange(
                        "(s p) t -> p s t", p=128))
                xhp.append(xh_t)
                xl_t = xp_p.tile([128, 2, TS], f8, tag=f"xl{j}")
                nc.sync.dma_start(
                    out=xl_t,
                    in_=xld[256 * j:256 * (j + 1), t0:t0 + TS].rearrange(
                        "(s p) t -> p s t", p=128))
                xlp.append(xl_t)
                if m4 == 0:
                    nc.sync.dma_start(
                        out=wk_sb[:, 2 * j:2 * j + 2, :, :],
                        in_=wkd[256 * j:256 * (j + 1), :, :].rearrange(
                            "(s p) a m -> p s a m", p=128))
                    nc.sync.dma_start(
                        out=wv_sb[:, 2 * j:2 * j + 2, :, :],
                        in_=wvd[256 * j:256 * (j + 1), :, :].rearrange(
                            "(s p) a m -> p s a m", p=128))
            cc_sl = tab_p.tile([128, TS], bf16, tag="cc")
            nc.sync.dma_start(out=cc_sl, in_=ccd[:, t0:t0 + TS])
            ss_sl = tab_p.tile([128, TS], bf16, tag="ss")
            nc.sync.dma_start(out=ss_sl, in_=ssd[:, t0:t0 + TS])
            ve_sl = tab_p.tile([128, TPS, HD], bf16, tag="ve")
            nc.sync.dma_start(
                out=ve_sl, in_=ved[t0:t0 + TS, :].rearrange("(tt p) h -> p tt h", p=128)
            )

            # ---- gate row: 3*sigmoid(x[:, :12] @ wg / (SX*SW)) ----
            ps_g = psQ.tile([128, TS], f32, tag="q")
            nc.tensor.matmul(ps_g[0:1, :], wg_sb, xhp[0][0:GATE_CH, 0, :],
                             start=True, stop=True)
            g_row = row_p.tile([1, TS], f32, tag="grow")
            nc.scalar.activation(g_row, ps_g[0:1, :], AF.Exp, scale=-DQ)
            nc.vector.tensor_scalar(out=g_row, in0=g_row, scalar1=1.0,
                                    scalar2=None, op0=OP.add)
            nc.vector.reciprocal(g_row, g_row)
            g_dr = dram_p.tile([TS], f32, tag="gdr")
            nc.sync.dma_start(out=g_dr, in_=g_row)
            gate_c = row_p.tile([128, TPS], f32, tag="gate")
            nc.sync.dma_start(
                out=gate_c,
                in_=bass.AP(tensor=g_dr.tensor, offset=g_dr.offset,
                            ap=[[1, 128], [128, TPS]]),
            )

            # ---- k projection + norm + rope ----
            ps_k = psQ.tile([128, TS], f32, tag="q")
            hilo_matmuls(ps_k, wk_sb, slice(0, HD), xhp, xlp, slice(0, TS))
            sq_k = sq_p.tile([128, TS], bf16, tag="sq")
            nc.scalar.activation(sq_k, ps_k, AF.Square)
            ps_nk = psQ.tile([128, TS], f32, tag="q")
            nc.tensor.matmul(ps_nk, ones_sq, sq_k, start=True, stop=True)
            lnk = sq_p.tile([128, TS], bf16, tag="ln")
            nc.scalar.activation(lnk, ps_nk, AF.Ln, bias=bk_sb, scale=S_K)
            rnk = nrm_p.tile([128, TS], bf16, tag="rn")
            nc.scalar.activation(rnk, lnk, AF.Exp, scale=-0.5)
            k_sl = kT_sb[:, t0:t0 + TS]
            nc.vector.tensor_mul(k_sl, ps_k, rnk)
            rope_inplace(k_sl, cc_sl, ss_sl, nc.vector)

            # ---- v projection (natural layout) + gate-add ----
            for tt in range(TPS):
                ps_v = psQ.tile([128, TS], f32, tag="q")
                hilo_matmuls_vnat(ps_v[:, 0:HD], xhp, xlp, tt, wv_sb)
                gtmp = wk_p.tile([128, HD], bf16, tag="gtmp")
                nc.vector.tensor_scalar(out=gtmp, in0=ve_sl[:, tt, :],
                                        scalar1=gate_c[:, tt:tt + 1], scalar2=3.0,
                                        op0=OP.mult, op1=OP.mult)
                nc.vector.scalar_tensor_tensor(
                    out=vn_sb[:, m4 * TPS + tt, :], in0=ps_v[:, 0:HD],
                    scalar=DQ, in1=gtmp, op0=OP.mult, op1=OP.add)

            # ---- q projections (4 heads) + norm + rope ----
            if m4 == 0:
                for j in range(NPR):
                    nc.sync.dma_start(
                        out=wq_sb[:, 2 * j:2 * j + 2, :, :],
                        in_=wqd[256 * j:256 * (j + 1), :, :].rearrange(
                            "(s p) a m -> p s a m", p=128))
            qts = []
            for h in range(HPC):
                ps_q = psQ.tile([128, TS], f32, tag="q")
                hilo_matmuls(ps_q, wq_sb, slice(h * HD, (h + 1) * HD),
                             xhp, xlp, slice(0, TS))
                sq_q = sq_p.tile([128, TS], bf16, tag="sq")
                nc.scalar.activation(sq_q, ps_q, AF.Square)
                ps_nq = psQ.tile([128, TS], f32, tag="q")
                nc.tensor.matmul(ps_nq, ones_sq, sq_q, start=True, stop=True)
                lnq = sq_p.tile([128, TS], bf16, tag="ln")
                nc.scalar.activation(lnq, ps_nq, AF.Ln, bias=bq_sb, scale=S_Q)
                rbc = nrm_p.tile([128, TS], bf16, tag="rn")
                nc.scalar.activation(rbc, lnq, AF.Exp, scale=-0.5)
                qt = qt_p.tile([128, TS], bf16, tag=f"qt{h}")
                nc.vector.tensor_mul(qt, ps_q, rbc)
                rope_inplace(qt, cc_sl, ss_sl, nc.vector)
                qts.append(qt)

            # ---- attention (scores transposed: tk on partitions, tq free) ----
            pairs = _pairs(m4, W)
            yts = []
            for h in range(HPC):
                ps_out = psO.tile([128, TS], f32, tag="o")
                ps_den = psD.tile([128, TS], f32, tag="d")
                nmem = sum(len(p[0]) for p in pairs)
                seen = 0
                for pi, (mem, u0, u1) in enumerate(pairs):
                    pss = psS.tile([128, 2, TS], f32, tag="sp")
                    for (s, n, f0, f1, cb, eb) in mem:
                        dst = pss[:, s, u0:u1]
                        # scores over the union extent, then -100 bias blocks
                        biases = []
                        if u0 < f0:
                            for b0 in range(u0, f0, 128):
                                biases.append((b0, fneg_sb))
                        if cb is not None:
                            biases.append((cb, btri_sb))
                        if eb is not None:
                            biases.append((eb, etri_sb))
                        if f1 < u1:
                            for b0 in range(f1, u1, 128):
                                biases.append((b0, fneg_sb))
                        nc.tensor.matmul(dst, kT_sb[:, n * 128:(n + 1) * 128],
                                         qts[h][:, u0:u1], start=True,
                                         stop=(not biases))
                        for bi, (b0, btile) in enumerate(biases):
                            nc.tensor.matmul(
                                pss[:, s, b0:b0 + 128], ident, btile,
                                start=False, stop=(bi == len(biases) - 1),
                                skip_group_check=True)
                    es = es_p.tile([128, 2, TS], bf16, tag="es")
                    if len(mem) == 2:
                        nc.scalar.activation(es[:, :, u0:u1], pss[:, :, u0:u1],
                                             AF.Exp)
                    else:
                        nc.scalar.activation(es[:, 0, u0:u1], pss[:, 0, u0:u1],
                                             AF.Exp)
                    for (s, n, f0, f1, cb, eb) in mem:
                        nc.tensor.matmul(ps_den[:, u0:u1], ones_sq,
                                         es[:, s, u0:u1],
                                         start=(seen == 0), stop=(seen == nmem - 1),
                                         skip_group_check=(seen > 0))
                        nc.tensor.matmul(ps_out[:, u0:u1], vn_sb[:, n, :],
                                         es[:, s, u0:u1],
                                         start=(seen == 0), stop=(seen == nmem - 1),
                                         skip_group_check=(seen > 0))
                        seen += 1
                sbc = sbc_p.tile([128, TS], f32, tag="sbc")
                nc.vector.reciprocal(sbc, ps_den)
                yt = y_p.tile([128, 2, TS], bf16, tag=f"y{h // 2}")
                nc.vector.tensor_mul(yt[:, h % 2, :], ps_out, sbc)
                yts.append(yt)

            # ---- c_proj partial (bf16): outT[co,t] = sum_h wpT[h].T @ y[h] ----
            if m4 == 0:
                for h in range(HPC):
                    nc.sync.dma_start(out=wp_sb[:, h, :],
                                      in_=wpd[h * 128:(h + 1) * 128, :])
            for cop in range(NTT // 2):
                ps_p = psS.tile([128, 2, TS], f32, tag="sp")
                for s in range(2):
                    co = 2 * cop + s
                    for h in range(HPC):
                        nc.tensor.matmul(
                            ps_p[:, s, :], wp_sb[:, h, co * 128:(co + 1) * 128],
                            yts[h][:, h % 2, :], start=(h == 0), stop=(h == HPC - 1),
                            skip_group_check=(s == 1))
                ot = ot_p.tile([128, 2, TS], bf16, tag="ot")
                nc.vector.tensor_copy(ot, ps_p)
                nc.sync.dma_start(
                    out=outT[256 * cop:256 * (cop + 1), t0:t0 + TS].rearrange(
                        "(s p) t -> p s t", p=128),
                    in_=ot)

    # Restrict the activation-table picker to the one set containing every
    # ACT function we use (exp, ln, square, copy, identity).
    import concourse.hw_specs as hw_specs
    import concourse.bacc as bacc_mod

    orig = hw_specs.get_activation_tables

    def only_combined(arch):
        t = orig(arch)
        return {k: (v if k == "natural_log_exp_and_others" else set())
                for k, v in t.items()}

    hw_specs.get_activation_tables = only_combined
    bacc_mod.get_activation_tables = only_combined
    try:
        nc.compile()
    finally:
        hw_specs.get_activation_tables = orig
        bacc_mod.get_activation_tables = orig
    return nc


def _hilo(a, scale):
    """Split scale*a into e4m3 hi + lo planes (returned as e4m3 arrays)."""
    s = (np.float32(scale) * a).astype(np.float32)
    hi = s.astype(E4np)
    lo = (s - hi.astype(np.float32)).astype(E4np)
    return hi, lo


def _prep_inputs(x, ve, cos, sin, Wq, Wk, Wv, Wproj, Wgate, W):
    cosT = np.ascontiguousarray(cos[0, :, 0, :].T)  # (64, T)
    sinT = np.ascontiguousarray(sin[0, :, 0, :].T)
    cc = np.concatenate([cosT, cosT], axis=0).astype(BFnp)
    ss = np.concatenate([sinT, -sinT], axis=0).astype(BFnp)
    p = np.arange(128)[:, None]
    f = np.arange(128)[None, :]
    # scores psum is [tk (partitions), tq (cols)]: bias[i, j] masks key i vs
    # query j. causal block: invalid when j < i; edge block: valid j <= i + W%128
    btri = np.where(f >= p, 0.0, NEG).astype(BFnp)
    etri = np.where(f <= p + (W % 128), 0.0, NEG).astype(BFnp)
    fneg = np.full((128, 128), NEG, dtype=np.float32).astype(BFnp)
    identb = np.eye(128, dtype=np.float32).astype(BFnp)

    in_maps = []
    for core in range(8):
        b, g = core // NKV, core % NKV
        hs = slice(g * HPC * HD, (g + 1) * HPC * HD)
        ks = slice(g * HD, (g + 1) * HD)
        xT = np.ascontiguousarray(x[b].T)
        xh, xl = _hilo(xT, SX)
        wqh, wql = _hilo(np.ascontiguousarray(Wq[hs, :].T), SW)
        wkh, wkl = _hilo(np.ascontiguousarray(Wk[ks, :].T), SW)
        wvh, wvl = _hilo(np.ascontiguousarray(Wv[ks, :].T), SW)
        wgh = (np.float32(SW) * Wgate[g][:, None]).astype(E4np)
        in_maps.append({
            "xh": xh,
            "xl": xl,
            "wq8": np.ascontiguousarray(np.stack([wqh, wql], axis=1)),
            "wk8": np.ascontiguousarray(np.stack([wkh, wkl], axis=1)),
            "wv8": np.ascontiguousarray(np.stack([wvh, wvl], axis=1)),
            "wpT": np.ascontiguousarray(Wproj[:, hs].T).astype(BFnp),
            "wg": np.ascontiguousarray(wgh),
            "cc": cc,
            "ss": ss,
            "ve": np.ascontiguousarray(ve[b][:, ks]).astype(BFnp),
            "btri": btri,
            "etri": etri,
            "identb": identb,
            "fneg": fneg,
        })
    return in_maps


def _run(inputs, trace=False):
    from concourse.bass_utils import run_bass_kernel_spmd

    x = np.asarray(inputs["x"], dtype=np.float32)
    ve = np.asarray(inputs["ve"], dtype=np.float32)
    cos = np.asarray(inputs["cos"], dtype=np.float32)
    sin = np.asarray(inputs["sin"], dtype=np.float32)
    Wq = np.asarray(inputs["Wq"], dtype=np.float32)
    Wk = np.asarray(inputs["Wk"], dtype=np.float32)
    Wv = np.asarray(inputs["Wv"], dtype=np.float32)
    Wproj = np.asarray(inputs["Wproj"], dtype=np.float32)
    Wgate = np.asarray(inputs["Wgate"], dtype=np.float32)
    W = int(inputs["window_size"])

    if W not in _compiled:
        _compiled[W] = _build(W)
    nc = _compiled[W]

    in_maps = _prep_inputs(x, ve, cos, sin, Wq, Wk, Wv, Wproj, Wgate, W)
    res = run_bass_kernel_spmd(nc, in_maps, core_ids=list(range(8)), trace=trace)

    out = np.zeros((B, T, C), dtype=np.float32)
    for core in range(8):
        b = core // NKV
        out[b] += res.results[core]["outT"].astype(np.float32).T
    return out, res


def kernel(**inputs):
    out, _ = _run(inputs, trace=False)
    return out
